# revision 1
# baseline (speedup 1.0000x reference)
"""Depth-to-space (CRD order) kernel for Trainium2, 8 NeuronCores.

in:  (32, 9, 512, 512) f32, channel c = r*3+s encodes (row_off, col_off)
out: (32, 1, 1536, 1536) f32 with out[b,0,3i+r,3j+s] = in[b,3r+s,i,j]

Sharding: data-parallel over batch, 4 batches per core, no communication.
Per core per (batch, 128-row chunk, row-offset r):
  - DMA-in  x[b, 3r:3r+3, i0:i0+128, :] -> SBUF [128, 3*512]    (768 KB,
    SP HWDGE ring; partition = image row, 2KB runs)
  - one strided-AP DVE copy interleaving the 3 channels into contiguous
    output rows: out[p, 3j+s] = in[p, s*512+j]
  - DMA-out [128, 1536] -> y rows 3*i0+r .. stride 3             (768 KB,
    ACT HWDGE ring; 6KB contiguous runs)
Loads and stores ride separate HWDGE rings so neither blocks the other
(FIFO per ring); measured ~197 us/core = ~94% of the 435 GB/s per-core
SBUF-port ceiling incl. ~11 us fixed NEFF preamble.
"""

import sys

import numpy as np

_B, _C, _H, _W = 32, 9, 512, 512
_K = 3
_NCORES = 8
_BLOC = _B // _NCORES  # 4

_PROG = None


def _ensure_path():
    try:
        import concourse.bass  # noqa: F401
    except ImportError:
        sys.path.insert(0, "/opt/trn_rl_repo")


def _build():
    import concourse.bacc as bacc
    import concourse.mybir as mybir
    from concourse import tile

    f32 = mybir.dt.float32
    nc = bacc.Bacc(None)
    x = nc.declare_dram_parameter("x", [_BLOC, _C, _H, _W], f32, isOutput=False)
    y = nc.declare_dram_parameter("y", [_BLOC, _K * _H, _K * _W], f32, isOutput=True)

    P = 128
    KW = _K * _W  # 1536

    with tile.TileContext(nc) as tc:
        with (
            tc.tile_pool(name="tin", bufs=6) as pin,
            tc.tile_pool(name="tout", bufs=6) as pout,
        ):
            su = 0
            for b in range(_BLOC):
                for i0 in range(0, _H, P):
                    # output rows 3*i0 .. 3*i0+384, grouped by row offset r
                    dst = y[b, _K * i0 : _K * (i0 + P), :].rearrange(
                        "(p r) w -> r p w", r=_K
                    )
                    for r in range(_K):
                        # dedicated HWDGE rings: SP carries loads, ACT stores;
                        # mixing them on one ring lets a not-yet-ready store
                        # block ready loads behind it (FIFO per ring). The
                        # edges are safe exceptions: first loads ride the
                        # still-idle store ring, last stores the drained load
                        # ring (no younger work queues behind them there).
                        ld_eng = nc.scalar if su < 2 else nc.sync
                        st_eng = nc.sync if su >= 46 else nc.scalar
                        su += 1
                        # copy r consumes exactly channels 3r..3r+2
                        tin = pin.tile([P, KW], f32)
                        ld_eng.dma_start(
                            out=tin[:].rearrange("p (s j) -> p s j", s=_K),
                            in_=x[b, _K * r : _K * (r + 1), i0 : i0 + P, :].rearrange(
                                "s p j -> p s j"
                            ),
                        )
                        # out[p, 3j+s] = in[p, s*512+j]
                        tout = pout.tile([P, KW], f32)
                        nc.vector.tensor_copy(
                            out=tout[:].rearrange("p (j s) -> p j s", s=_K),
                            in_=tin[:].rearrange("p (s j) -> p j s", s=_K),
                        )
                        st_eng.dma_start(out=dst[r], in_=tout[:])
    return nc


def _run(x_full, trace=False, **spmd_kwargs):
    """x_full: (32, 9, 512, 512) f32 ndarray. Returns (out, BassKernelResults)."""
    global _PROG
    _ensure_path()
    from concourse.bass_utils import run_bass_kernel_spmd

    if _PROG is None:
        _PROG = _build()
        if not _PROG.is_finalized():
            _PROG.finalize()
    in_maps = [
        {"x": np.ascontiguousarray(x_full[i * _BLOC : (i + 1) * _BLOC])}
        for i in range(_NCORES)
    ]
    res = run_bass_kernel_spmd(
        _PROG, in_maps, core_ids=list(range(_NCORES)), trace=trace, **spmd_kwargs
    )
    out = np.concatenate([np.asarray(r["y"]) for r in res.results], axis=0)
    return out.reshape(_B, 1, _K * _H, _K * _W), res


def kernel(**inputs):
    x = np.ascontiguousarray(np.asarray(inputs["inputs"], dtype=np.float32))
    k = int(np.asarray(inputs.get("kernel_size", _K)))
    assert k == _K, f"kernel hardcodes kernel_size=3, got {k}"
    assert x.shape == (_B, _C, _H, _W), x.shape
    out, _ = _run(x)
    return out



# revision 2
# speedup vs baseline: 1.2072x; 1.2072x over previous
"""Depth-to-space (CRD order) kernel for Trainium2, 8 NeuronCores.

in:  (32, 9, 512, 512) f32, channel c = r*3+s encodes (row_off, col_off)
out: (32, 1, 1536, 1536) f32 with out[b,0,3i+r,3j+s] = in[b,3r+s,i,j]

Sharding: data-parallel over batch, 4 batches per core, no communication.

Design notes (HW-measured on trn2):
- The 16 SDMA engines are the bottleneck; they stream descriptors serially
  and their per-engine payload rate depends on the per-partition contiguous
  run length: HBM reads ~23.2 / 25.9 / 26.8 GB/s at 2K/8K/32K runs, HBM
  writes ~26.2-26.8 at >=6K runs.  Engine busy time tracks the LARGER side
  of each descriptor, so bf16-in-DMA cast (SWDGE) does not help: the f32
  HBM side still bounds it (verified: cast stores' busy time == f32 ones).
- Per-core layout: partition p holds image rows 4p..4p+3, so loads pull a
  channel-triple with 8KB contiguous runs and stores write output
  row-triples (3*(4p+d)+{0,1,2}) as 18KB contiguous runs.
- Loads ride the sync HWDGE ring, stores the scalar ring (separate FIFOs so
  a waiting store never blocks ready loads); DVE interleaves in between.
- ~9us fixed preamble (all-engine sem rendezvous etc) + ~179us DMA floor.
  Measured 192.4us/core on a quiet device (baseline fine-grained kernel:
  197.9us).  Keep DMA-issuing engines free of compute ops (ACT copies on
  the scalar ring head-of-line-block store dispatch), and keep the
  per-(d) copy->store chains in issue order (clustering stores at the
  batch end stalls the pipeline).
"""

import sys

import numpy as np

_B, _C, _H, _W = 32, 9, 512, 512
_K = 3
_NCORES = 8
_BLOC = _B // _NCORES  # 4

_PROG = None


def _ensure_path():
    try:
        import concourse.bass  # noqa: F401
    except ImportError:
        sys.path.insert(0, "/opt/trn_rl_repo")


def _build():
    import concourse.bacc as bacc
    import concourse.mybir as mybir
    from concourse import tile

    f32 = mybir.dt.float32
    nc = bacc.Bacc(None)
    x = nc.declare_dram_parameter("x", [_BLOC, _C, _H, _W], f32, isOutput=False)
    y = nc.declare_dram_parameter("y", [_BLOC, _K * _H, _K * _W], f32, isOutput=True)

    P = 128
    RP = 4  # image rows per partition
    KW = _K * _W  # 1536

    with tile.TileContext(nc) as tc:
        with (
            tc.tile_pool(name="tin", bufs=5) as pin,
            tc.tile_pool(name="tout", bufs=4) as pout,
        ):
            for b in range(_BLOC):
                tins = []
                for g in range(_K):
                    tin = pin.tile([P, _K * RP * _W], f32, name="tin")
                    nc.sync.dma_start(
                        out=tin[:].rearrange("p (c dj) -> p c dj", c=_K),
                        in_=x[b, _K * g : _K * (g + 1), :, :].rearrange(
                            "c (p d) j -> p c (d j)", d=RP
                        ),
                    )
                    tins.append(tin[:].rearrange("p (c d j) -> p c d j", c=_K, d=RP))
                ydst = y[b, :, :].rearrange("(p q r) w -> q p (r w)", q=RP, r=_K)
                for d in range(RP):
                    tout = pout.tile([P, _K * KW], f32, name="tout")
                    for r in range(_K):
                        # out[p, 3j+s] = x[b, 3r+s, 4p+d, j]
                        nc.vector.tensor_copy(
                            out=tout[:, r * KW : (r + 1) * KW].rearrange(
                                "p (j s) -> p j s", s=_K
                            ),
                            in_=tins[r][:, :, d, :].rearrange("p s j -> p j s"),
                        )
                    nc.scalar.dma_start(out=ydst[d], in_=tout[:])
    return nc


def _run(x_full, trace=False, **spmd_kwargs):
    """x_full: (32, 9, 512, 512) f32 ndarray. Returns (out, BassKernelResults)."""
    global _PROG
    _ensure_path()
    from concourse.bass_utils import run_bass_kernel_spmd

    if _PROG is None:
        _PROG = _build()
        if not _PROG.is_finalized():
            _PROG.finalize()
    in_maps = [
        {"x": np.ascontiguousarray(x_full[i * _BLOC : (i + 1) * _BLOC])}
        for i in range(_NCORES)
    ]
    res = run_bass_kernel_spmd(
        _PROG, in_maps, core_ids=list(range(_NCORES)), trace=trace, **spmd_kwargs
    )
    out = np.concatenate([np.asarray(r["y"]) for r in res.results], axis=0)
    return out.reshape(_B, 1, _K * _H, _K * _W), res


def kernel(**inputs):
    x = np.ascontiguousarray(np.asarray(inputs["inputs"], dtype=np.float32))
    k = int(np.asarray(inputs.get("kernel_size", _K)))
    assert k == _K, f"kernel hardcodes kernel_size=3, got {k}"
    assert x.shape == (_B, _C, _H, _W), x.shape
    out, _ = _run(x)
    return out
